# revision 68
# baseline (speedup 1.0000x reference)
"""Distributed Trainium2 Bass kernel for nn_Attention_62766652063769.

Reference computation (B=4, T=2048, C=1024, H=16, HD=64):
    qkv = x @ W_qkv^T ; split into q, k, v heads
    q, k <- RoPE(q), RoPE(k)   (interleaved-pair rotation)
    attn = softmax(q k^T / sqrt(HD))   (mask is all-ones -> no masking)
    out  = (attn @ v) @ W_proj^T

Sharding (tensor-parallel head split, per the problem hint): core c owns
batch b = c//2 and head half c%2 (8 of 16 heads), ALL 2048 q tokens.
K/V are computed only for the owned heads - no redundant compute and no
device-to-device communication.  The row-parallel out-projection yields
partial sums; the host adds the two head-halves per batch while
unsharding (the standard TP epilogue).

Schedule: one software-pipelined stream.  The softmax exp runs on the
ACT engine (the per-core floor: 8 heads x 2048 x 2048 = 33.5M exps
~ 293us); everything else is arranged so the PE stays dense (HAM stays
warm) and near its own ~345us stream floor:

  - attention is blocked per head-pair x tq-512-chunk: score tile ST
    [tk=128, headA 512 | headB 512] -> one ACTIVATE(exp) of FD=1024 ->
    OT accumulation [65, 512] per head over the 16 k-tiles (65th V
    column = ones gives the softmax denominator).
  - zero-padded Q trick: Q stored as [128, 2, T] with head A dims on
    partitions 0:64 of plane 0 (zeros elsewhere) and head B on 64:128
    of plane 1, so BOTH heads' score matmuls use the same full-K=128
    stationary K-tile (the zero rows annihilate the cross-head terms).
    This makes every matmul in the kernel a plain K=128/N=512 stream
    that pipelines back-to-back at ~216ns with LDWEIGHTS hidden
    (tile_position row-packing does NOT overlap on this toolchain).
  - OT matmuls for iteration i-1 are emitted after ST/ACT of iteration
    i so the in-order PE queue never waits on the ACT.
  - QK for pair p+1 and the V tiles are chopped into ~1us ticks and
    interleaved into pair p's 64 attention iterations; the out-proj is
    interleaved into the last pair's iterations, gated per completed
    token quarter.
  - PSUM budget: ST 2 slots x 2 banks + OT psA/psB 1 bank each +
    2 x 1-bank fill slots for the interleaved QKV/proj chunks = 8.
  - epilogue per (pair, tq-chunk): PSUM released by 4 DVE copies;
    1/denom via DRAM-fold to [128, 8] + DVE InstReciprocal (keeps the
    saturated ACT queue out of the epilogue); free-axis broadcast via
    DRAM roundtrip; normalize on DVE.
  - DMA dispatch costs ~600ns of issuing-engine (SP/ACT HWDGE) queue
    time, so lead-in bulk loads go on the idle ACT queue; GPSIMD SWDGE
    dispatch is slower - only the rope add lives on GPSIMD.

RoPE on-chip: the per-head feature permutation even/odd -> halves is
folded into W_q/W_k rows on the host, so the rotation becomes
    out = cos*X + swap32(sinB*X)
with straight 32-row block swaps (done by SBUF-to-SBUF DMA).

bf16 matmuls (fp32 PSUM accumulate).  Minimal lead-in: only the first
two chunks of Q(0)/K(0)/V(0) are emitted before the attention stream
starts; the rest front-pace into pair 0's iterations as WHOLE-chunk
fill ticks (a chunk's psum-slot readers are emitted right after its
matmuls, which keeps fill-slot WAR deps acyclic - split ticks deadlock
the Tile scheduler).  Lead-in chunks borrow the idle 'st' PSUM slots;
the final epilogue uses the ACT Ln/Exp reciprocal (ACT idle by then);
the post-loop out-proj remainder is split so its pair-0..2 accumulation
matmuls overlap the final epilogue's chain (only the pp=3 matmuls are
gated on it).  Measured ~403-410us HW exec (~1.47x over the 595us
baseline; run-to-run device variance up to ~+-40us), rel err ~9.4e-3.
"""

import os
import re
import sys
import types

if "/opt/trn_rl_repo" not in sys.path:
    sys.path.insert(0, "/opt/trn_rl_repo")

import ml_dtypes
import numpy as np

import bass_rust
import concourse.bass as bass
import concourse.mybir as mybir
from concourse import bass_utils
from concourse.tile import TileContext, ScopedClock

# ---------------------------------------------------------------------------
# Environment patches
# ---------------------------------------------------------------------------

def _patched_drain_and_barrier(self, tick_clock, wait_clock):
    """The walrus build in this container encodes at most one sync-wait per
    instruction; Tile's tail drain carries one wait per live semaphore.
    Emit single-wait NOPs on SP instead, then an unguarded drain."""
    gc = tick_clock.global_clock
    ticks = [int(x) for x in re.findall(r"\d+", repr(gc))]
    for i, t in enumerate(ticks):
        if t <= 0:
            continue
        l = [0] * len(ticks)
        l[i] = t
        nop = self.nc.sync.nop(nofuse=True)
        wait_clock.add_sem_waits(nop.ins, ScopedClock({None: bass_rust.VectorClock(l)}))
    self.nc.sync.drain()
    self.nc.all_engine_barrier()
    assert self.sems is not None
    popped = self.nc._tile_sem_poison_stack.pop()
    assert popped is self._sem_poison
    self.nc.clear_and_free_semaphores(list(self.sems.allocated().values()))
    self.nc.all_engine_barrier()


TileContext._drain_and_barrier = _patched_drain_and_barrier


def _split_multi_waits(nc):
    """Move extra sync-waits onto single-wait NOPs inserted just before the
    owning instruction on the same (in-order) engine."""
    for func in nc.m.functions:
        for bb in func.blocks:
            insts = bb.instructions
            if not any(
                i.sync_info is not None
                and i.sync_info.on_wait
                and len(i.sync_info.on_wait) > 1
                for i in insts
            ):
                continue
            new = []
            for inst in insts:
                si = inst.sync_info
                if si is not None and si.on_wait and len(si.on_wait) > 1:
                    waits = list(si.on_wait)
                    for w in waits[:-1]:
                        nop = mybir.InstNoOp(
                            name=nc.get_next_instruction_name(),
                            engine=inst.engine,
                            bass_nofuse=True,
                            sync_info=mybir.SyncInfo(on_wait=[w], on_update=[]),
                        )
                        nc.register_instruction(nop)
                        new.append(nop)
                    inst.sync_info = mybir.SyncInfo(
                        on_wait=[waits[-1]], on_update=list(si.on_update)
                    )
                new.append(inst)
            bb.instructions = new


def _install_ntff_hook():
    """Recreate antenv.axon_hooks (absent in this image) so
    run_bass_kernel_spmd(trace=True) can profile through libaxon_pjrt."""
    if "antenv.axon_hooks" in sys.modules:
        return
    import contextlib
    import ctypes

    mod = types.ModuleType("antenv.axon_hooks")
    _state = {"hook": None}

    def set_axon_ntff_profile_hook(hook):
        _state["hook"] = hook

    def get_axon_ntff_profile_hook():
        return _state["hook"]

    def _ntff_profile_via_ctypes(so_path):
        lib = ctypes.CDLL(so_path)
        if not hasattr(lib, "axon_start_nrt_profile"):
            return None
        lib.axon_start_nrt_profile.argtypes = [
            ctypes.POINTER(ctypes.c_int64),
            ctypes.c_size_t,
        ]
        lib.axon_start_nrt_profile.restype = ctypes.c_int64
        lib.axon_stop_nrt_profile.argtypes = [ctypes.c_char_p]
        lib.axon_stop_nrt_profile.restype = ctypes.c_int64

        @contextlib.contextmanager
        def _hook(output_dir, device_ids):
            import jax

            jax.devices()
            if device_ids:
                ids = (ctypes.c_int64 * len(device_ids))(*device_ids)
                rc = lib.axon_start_nrt_profile(ids, len(device_ids))
            else:
                rc = lib.axon_start_nrt_profile(None, 0)
            if rc != 0:
                raise RuntimeError(f"axon_start_nrt_profile rc={rc}")
            try:
                yield
            finally:
                n = lib.axon_stop_nrt_profile(str(output_dir).encode())
                if n < 0:
                    raise RuntimeError(f"axon_stop_nrt_profile rc={n}")
                print(f"profile: {n} file(s) in {output_dir}", file=sys.stderr)

        return _hook

    mod.set_axon_ntff_profile_hook = set_axon_ntff_profile_hook
    mod.get_axon_ntff_profile_hook = get_axon_ntff_profile_hook
    try:
        set_axon_ntff_profile_hook(
            _ntff_profile_via_ctypes("/opt/axon/libaxon_pjrt.so")
        )
    except Exception:
        pass
    sys.modules["antenv.axon_hooks"] = mod
    try:
        import antenv

        antenv.axon_hooks = mod
    except ImportError:
        pass


_install_ntff_hook()


# ---------------------------------------------------------------------------
# Problem constants
# ---------------------------------------------------------------------------

B, T, C = 4, 2048, 1024
H, HD = 16, 64
NCORES = 8
NPAIR = H // 2       # global head pairs (=8)
NP_CORE = 4          # head pairs owned per core (tensor-parallel head split)
KT_TILES = T // 128  # 16
NTQH = T // 512      # q-token 512-chunks per core (= 4; core owns all of T)
SCALE = 1.0 / np.sqrt(HD)

F32 = mybir.dt.float32
BF16 = mybir.dt.bfloat16

CC = C // 128   # contraction chunks for qkv projections (= 8)
CC2 = CC // 2   # contraction chunks for the row-parallel out-proj (= 4)


# ---------------------------------------------------------------------------
# Device program
# ---------------------------------------------------------------------------

def _build_nc():
    nc = bass.Bass(trn_type="TRN2", target_bir_lowering=False, debug=False)

    xt = nc.declare_dram_parameter("xt", [C, T], BF16, isOutput=False)
    wqt = nc.declare_dram_parameter("wqt", [NP_CORE, 128, CC, 128], BF16,
                                    isOutput=False)
    wkt = nc.declare_dram_parameter("wkt", [NP_CORE, 128, CC, 128], BF16,
                                    isOutput=False)
    wvt = nc.declare_dram_parameter("wvt", [128, CC, 512], BF16,
                                    isOutput=False)
    wpt = nc.declare_dram_parameter("wpt", [C // 2, C], BF16, isOutput=False)
    cosk = nc.declare_dram_parameter("cosk", [128, T], BF16, isOutput=False)
    sink = nc.declare_dram_parameter("sink", [128, T], BF16, isOutput=False)
    out_ext = nc.declare_dram_parameter("out", [T, C], F32, isOutput=True)

    rs_dram = nc.dram_tensor("rs_scratch", [NP_CORE, NTQH, 2, 512], F32)
    d_dram = nc.dram_tensor("d_scratch", [NP_CORE, NTQH, 2, 512], F32)

    with TileContext(nc) as tc:
        with tc.tile_pool(name="persist", bufs=1) as persist, \
             tc.tile_pool(name="stp", bufs=2, space="PSUM") as stp, \
             tc.tile_pool(name="fillp", bufs=2, space="PSUM") as fillp, \
             tc.tile_pool(name="otp", bufs=1, space="PSUM") as otp, \
             tc.tile_pool(name="ptp", bufs=3) as ptp, \
             tc.tile_pool(name="wpool", bufs=2) as wpool, \
             tc.tile_pool(name="ropep", bufs=2) as ropep, \
             tc.tile_pool(name="kpool", bufs=2) as kpool, \
             tc.tile_pool(name="qpool", bufs=2) as qpool, \
             tc.tile_pool(name="epi1", bufs=1) as epi1, \
             tc.tile_pool(name="epi", bufs=2) as epi:

            att_sb = persist.tile([128, NP_CORE, T], BF16, tag="att")
            v_sb = persist.tile([128, KT_TILES, 8, 65], BF16, tag="v")
            pair_kt = {}  # pair -> rotating [128, T] K tile
            # pair -> rotating zero-padded Q tile [128, 2, T]:
            #   plane 0 rows 0:64   = head A dims, rows 64:128 = 0
            #   plane 1 rows 64:128 = head B dims, rows 0:64   = 0
            # With K stored as [A dims; B dims] on 128 partitions, the score
            # matmul for either head uses the SAME full-K=128 stationary
            # K-tile (the zero rows annihilate the other head), so the two
            # matmuls pipeline back-to-back with one LDWEIGHTS and no
            # tile_position.
            pair_qt = {}
            xt_a = persist.tile([128, CC // 2, T], BF16, tag="xta")
            xt_b = persist.tile([128, CC // 2, T], BF16, tag="xtb")
            ck = persist.tile([128, T], BF16, tag="ck")
            sk = persist.tile([128, T], BF16, tag="sk")
            wp_sb = persist.tile([128, CC2, C], BF16, tag="wp")

            # Pair-0 weights first (small, unblock the first matmuls), then
            # xt in 512-token column chunks in consumption order, then rope
            # tables.  wp (proj weights) last - only needed at the end.
            # DMA dispatch costs ~600ns of issuing-engine queue time each,
            # so the lead-in loads are spread across the idle engines'
            # queues to land in parallel.
            wq0 = wpool.tile([128, CC, 128], BF16, tag="w")
            nc.sync.dma_start(out=wq0, in_=wqt[0])
            wk0 = wpool.tile([128, CC, 128], BF16, tag="w")
            nc.sync.dma_start(out=wk0, in_=wkt[0])
            xt_r = xt.rearrange("(cc p) t -> p cc t", p=128)
            wv0 = wpool.tile([128, CC, 512], BF16, tag="wv")
            for c in range(4):
                t0, t1 = c * 512, (c + 1) * 512
                nc.scalar.dma_start(
                    out=xt_a[:, :, t0:t1], in_=xt_r[:, 0:CC // 2, t0:t1])
                nc.sync.dma_start(
                    out=xt_b[:, :, t0:t1], in_=xt_r[:, CC // 2:CC, t0:t1])
                nc.sync.dma_start(out=ck[:, t0:t1], in_=cosk[:, t0:t1])
                nc.sync.dma_start(out=sk[:, t0:t1], in_=sink[:, t0:t1])
                if c == 1:
                    # 1MB wv load on the ACT queue after the first two xt_a
                    # chunks (V tiles are not needed until pair-0 starts)
                    nc.scalar.dma_start(out=wv0, in_=wvt[:, :, :])
            nc.scalar.dma_start(
                out=wp_sb, in_=wpt.rearrange("(cc p) e -> p cc e", p=128)
            )
            nc.vector.memset(v_sb[:, :, :, 64:65], 1.0)

            def _xt(cc):
                return (xt_a, xt_b)[cc // (CC // 2)][:, cc % (CC // 2), :]

            def _rope(ps, ct, st_tab, out_ap):
                """out = ct*ps + swap32(st_tab*ps); ps PSUM f32, out bf16."""
                u = ropep.tile([128, 512], BF16, tag="u")
                v = ropep.tile([128, 512], BF16, tag="v")
                vs = ropep.tile([128, 512], BF16, tag="vs")
                nc.vector.tensor_mul(u, ps, ct)
                nc.vector.tensor_mul(v, ps, st_tab)
                for blk in range(4):
                    r = blk * 32
                    s = (blk ^ 1) * 32
                    nc.sync.dma_start(out=vs[r:r + 32, :],
                                      in_=v[s:s + 32, :])
                nc.gpsimd.tensor_add(out_ap, u, vs)

            def gen_q(p, wq=None, alt_first=0):
                """Q projection+rope for pair p, in ~0.9us ticks.
                alt=True (lead-in only): even chunks borrow the idle
                'st' PSUM slots so four chunk-buffers are in flight and
                the PE never stalls on the rope chain releasing a slot.
                """
                if wq is None:
                    wq = wpool.tile([128, CC, 128], BF16, tag="w")
                    nc.sync.dma_start(out=wq, in_=wqt[p])
                    yield
                qtp = qpool.tile([128, 2, T], BF16, tag="qt")
                pair_qt[p] = qtp
                if p < 2:
                    # zero the pad halves once per slot; later pairs reuse
                    # the slot round-robin and the pads stay zero (rope DMAs
                    # below only ever touch the live halves)
                    nc.vector.memset(qtp[64:128, 0, :], 0.0)
                    nc.vector.memset(qtp[0:64, 1, :], 0.0)
                for c in range(NTQH):  # 512-wide chunks over all T q tokens
                    if c < alt_first and c % 2 == 0:
                        ps = stp.tile([128, 512], F32, tag="st")
                    else:
                        ps = fillp.tile([128, 512], F32, tag="fill")
                    # one COMPLETE chunk per tick: the psum slot's readers
                    # (rope muls) are emitted right after its matmuls, so
                    # fill-slot WAR deps can never invert across generators
                    for cc in range(CC):
                        nc.tensor.matmul(
                            ps, lhsT=wq[:, cc, :],
                            rhs=_xt(cc)[:, c * 512:(c + 1) * 512],
                            start=(cc == 0), stop=(cc == CC - 1),
                        )
                    qc = ropep.tile([128, 512], BF16, tag="qc")
                    _rope(ps, ck[:, c * 512:(c + 1) * 512],
                          sk[:, c * 512:(c + 1) * 512], qc)
                    nc.sync.dma_start(
                        out=qtp[0:64, 0, c * 512:(c + 1) * 512],
                        in_=qc[0:64, :])
                    nc.sync.dma_start(
                        out=qtp[64:128, 1, c * 512:(c + 1) * 512],
                        in_=qc[64:128, :])
                    yield

            def gen_k(p, wk=None, alt_first=0):
                """K projection+rope for pair p, in ~0.9us ticks."""
                if wk is None:
                    wk = wpool.tile([128, CC, 128], BF16, tag="w")
                    nc.sync.dma_start(out=wk, in_=wkt[p])
                    yield
                ktp = kpool.tile([128, T], BF16, tag="kt")
                pair_kt[p] = ktp
                for c in range(4):  # 512-wide chunks over all T k tokens
                    if c < alt_first and c % 2 == 0:
                        ps = stp.tile([128, 512], F32, tag="st")
                    else:
                        ps = fillp.tile([128, 512], F32, tag="fill")
                    for cc in range(CC):
                        nc.tensor.matmul(
                            ps, lhsT=wk[:, cc, :],
                            rhs=_xt(cc)[:, c * 512:(c + 1) * 512],
                            start=(cc == 0), stop=(cc == CC - 1),
                        )
                    _rope(ps, ck[:, c * 512:(c + 1) * 512],
                          sk[:, c * 512:(c + 1) * 512],
                          ktp[:, c * 512:(c + 1) * 512])
                    yield

            def gen_v(tt_range, wv=None, alt_first=0):
                """V projection for the core's 8 heads, weight-moving form
                (N=512) so the 107ns LDWEIGHTS hides under the streams."""
                if wv is None:
                    wv = wpool.tile([128, CC, 512], BF16, tag="wv")
                    nc.sync.dma_start(out=wv, in_=wvt[:, :, :])
                    yield
                for n_emitted, tt in enumerate(tt_range):
                    if n_emitted < alt_first and n_emitted % 2 == 0:
                        ps = stp.tile([128, 512], F32, tag="st")
                    else:
                        ps = fillp.tile([128, 512], F32, tag="fill")
                    for cc in range(CC):
                        nc.tensor.matmul(
                            ps, lhsT=_xt(cc)[:, tt * 128:(tt + 1) * 128],
                            rhs=wv[:, cc, :],
                            start=(cc == 0), stop=(cc == CC - 1),
                        )
                    nc.vector.tensor_copy(
                        v_sb[:, tt, :, 0:64],
                        ps.rearrange("p (h d) -> p h d", h=8),
                    )
                    yield

            def emit_ot(p, kt, pt, psA, psB):
                nc.tensor.matmul(
                    psA[0:65, :], lhsT=v_sb[:, kt, 2 * p, :],
                    rhs=pt[:, 0:512],
                    start=(kt == 0), stop=(kt == KT_TILES - 1),
                )
                nc.tensor.matmul(
                    psB[0:65, :], lhsT=v_sb[:, kt, 2 * p + 1, :],
                    rhs=pt[:, 512:1024],
                    start=(kt == 0), stop=(kt == KT_TILES - 1),
                )

            def emit_epilogue(p, tqh, psA, psB, fast_tail=False):
                """Free PSUM fast (DVE copies only), 1/denom on DVE via a
                DRAM fold to [128, 8] (InstReciprocal is 8 cyc/elem, so give
                it all 128 lanes), broadcast via DRAM, normalize into
                att_sb.  Keeps the saturated ACT queue out of the epilogue
                entirely.  Engine ops keep partition offsets aligned (no
                cross-partition moves except via DMA)."""
                q0 = tqh * 512
                # unnormalized attn rows off PSUM
                osbA = epi.tile([64, 512], BF16, tag="osbA")
                osbB = epi.tile([64, 512], BF16, tag="osbB")
                nc.vector.tensor_copy(osbA, psA[0:64, :])
                nc.vector.tensor_copy(osbB, psB[0:64, :])
                if fast_tail:
                    # final epilogue: ACT is idle by now and its Ln/Exp
                    # path is ~3us shorter than the DMA-fold chain that
                    # the tail out-proj tiles wait on
                    rsl = epi.tile([128, 2, 512], F32, tag="dsb")
                    rsb = epi1.tile([128, 2, 512], F32, tag="rsb")
                    nc.scalar.activation(
                        out=rsl[64:65, 0, :], in_=psA[64:65, :],
                        func=mybir.ActivationFunctionType.Ln)
                    nc.scalar.activation(
                        out=rsl[64:65, 1, :], in_=psB[64:65, :],
                        func=mybir.ActivationFunctionType.Ln)
                    nc.scalar.activation(
                        out=rsb[64:65, :, :], in_=rsl[64:65, :, :],
                        func=mybir.ActivationFunctionType.Exp, scale=-1.0)
                    nc.sync.dma_start(
                        out=rs_dram[p, tqh], in_=rsb[64:65, :, :])
                else:
                    # denominator rows off PSUM (with the osb copies above,
                    # all that gates psA/psB release), then 1/denom on DVE
                    # via a DRAM fold to [128, 8]
                    dsb = epi.tile([128, 2, 512], F32, tag="dsb")
                    nc.vector.tensor_copy(dsb[64:65, 0, :], psA[64:65, :])
                    nc.vector.tensor_copy(dsb[64:65, 1, :], psB[64:65, :])
                    nc.sync.dma_start(
                        out=d_dram[p, tqh], in_=dsb[64:65, :, :])
                    dfold = epi.tile([128, 8], F32, tag="dfold")
                    nc.sync.dma_start(
                        out=dfold,
                        in_=d_dram[p, tqh].rearrange("a w -> (a w)")
                        .rearrange("(pp f) -> pp f", pp=128))
                    rfold = epi.tile([128, 8], F32, tag="rfold")
                    nc.vector.reciprocal(out=rfold, in_=dfold)
                    nc.sync.dma_start(
                        out=rs_dram[p, tqh].rearrange("a w -> (a w)")
                        .rearrange("(pp f) -> pp f", pp=128),
                        in_=rfold)
                bcA = epi.tile([64, 512], F32, tag="bcA")
                bcB = epi.tile([64, 512], F32, tag="bcB")
                nc.sync.dma_start(
                    out=bcA,
                    in_=rs_dram[p, tqh, 0:1, :].broadcast_to([64, 512]),
                )
                nc.sync.dma_start(
                    out=bcB,
                    in_=rs_dram[p, tqh, 1:2, :].broadcast_to([64, 512]),
                )
                nc.vector.tensor_mul(
                    att_sb[0:64, p, q0:q0 + 512], osbA, bcA)
                attB = epi.tile([64, 512], BF16, tag="attB")
                nc.vector.tensor_mul(attB, osbB, bcB)
                nc.sync.dma_start(
                    out=att_sb[64:128, p, q0:q0 + 512], in_=attB)

            def gen_proj():
                """Row-parallel out-proj in [128-token, 512-col] partial
                tiles; interleaved into pair 3's ACT-bound iterations
                (fill-tag PSUM is free there)."""
                for tt in range(T // 128):
                    for nch in range(2):
                        ps = fillp.tile([128, 512], F32, tag="fill")
                        for pp in range(NP_CORE):
                            nc.tensor.matmul(
                                ps,
                                lhsT=att_sb[:, pp, tt * 128:(tt + 1) * 128],
                                rhs=wp_sb[:, pp, nch * 512:(nch + 1) * 512],
                                start=(pp == 0), stop=(pp == NP_CORE - 1),
                            )
                        oh = epi.tile([128, 512], F32, tag="o")
                        nc.vector.tensor_copy(oh, ps)
                        nc.sync.dma_start(
                            out=out_ext[tt * 128:(tt + 1) * 128,
                                        nch * 512:(nch + 1) * 512],
                            in_=oh)
                        yield

            def emit_tail_proj(done):
                """Post-loop out-proj remainder, split so the pair-0..2
                accumulation matmuls run DURING the final epilogue's chain
                (they don't read pair 3's att); only the pp=3 matmuls are
                gated on it.  Up to 6 units in flight across the now-idle
                PSUM tags."""
                rem = [(u // 2, u % 2) for u in range(done, 2 * (T // 128))]
                wave = rem[:6]
                pools = [(fillp, "fill"), (fillp, "fill"), (stp, "st"),
                         (stp, "st"), (otp, "psA"), (otp, "psB")]
                tiles = []
                for (tt, nch), (pool, tag) in zip(wave, pools):
                    ps = pool.tile([128, 512], F32, tag=tag)
                    for pp in range(NP_CORE - 1):
                        nc.tensor.matmul(
                            ps, lhsT=att_sb[:, pp, tt * 128:(tt + 1) * 128],
                            rhs=wp_sb[:, pp, nch * 512:(nch + 1) * 512],
                            start=(pp == 0), stop=False)
                    tiles.append(ps)
                for (tt, nch), ps in zip(wave, tiles):
                    nc.tensor.matmul(
                        ps, lhsT=att_sb[:, 3, tt * 128:(tt + 1) * 128],
                        rhs=wp_sb[:, 3, nch * 512:(nch + 1) * 512],
                        start=False, stop=True)
                    oh = epi.tile([128, 512], F32, tag="o")
                    nc.vector.tensor_copy(oh, ps)
                    nc.sync.dma_start(
                        out=out_ext[tt * 128:(tt + 1) * 128,
                                    nch * 512:(nch + 1) * 512],
                        in_=oh)
                for tt, nch in rem[6:]:
                    ps = fillp.tile([128, 512], F32, tag="fill")
                    for pp in range(NP_CORE):
                        nc.tensor.matmul(
                            ps, lhsT=att_sb[:, pp, tt * 128:(tt + 1) * 128],
                            rhs=wp_sb[:, pp, nch * 512:(nch + 1) * 512],
                            start=(pp == 0), stop=(pp == NP_CORE - 1))
                    oh = epi.tile([128, 512], F32, tag="o")
                    nc.vector.tensor_copy(oh, ps)
                    nc.sync.dma_start(
                        out=out_ext[tt * 128:(tt + 1) * 128,
                                    nch * 512:(nch + 1) * 512],
                        in_=oh)

            # ---------------- lead-in: QK(0) + first V tiles --------------
            q0g = gen_q(0, wq0, alt_first=2)
            for _ in range(2):
                next(q0g)
            k0g = gen_k(0, wk0, alt_first=2)
            for _ in range(2):
                next(k0g)
            v0g = gen_v(range(0, KT_TILES), wv0, alt_first=2)
            for _ in range(2):
                next(v0g)

            # ---------------- main pair loop ------------------------------
            ITERS = [(tqh, kt) for tqh in range(NTQH)
                     for kt in range(KT_TILES)]
            NIT = len(ITERS)

            def gen_qk(p):
                yield from gen_q(p)
                yield from gen_k(p)

            for p in range(NP_CORE):
                # fill generators consumed during pair p's iterations:
                # [gen, total_ticks, due_fn(i), ticked]
                fills = []
                if p == 0:
                    # rest of pair 0's own K/V/Q, front-paced just ahead of
                    # their consumers (ST kt needs K chunk kt//4 by iter
                    # 4*(kt//4); OT kt needs V tile kt by iter kt+1; ST tqh
                    # needs Q chunk tqh by iter 16*tqh)
                    fills.append(
                        [k0g, 2, lambda i: max(0, (i + 2) // 4), 0])
                    fills.append([v0g, 14, lambda i: i + 1, 0])
                    fills.append([q0g, 2, lambda i: (i + 4) // 16, 0])
                if p + 1 < NP_CORE:
                    fills.append(
                        [gen_qk(p + 1), 10,
                         lambda i: -(-10 * (i + 1) // NIT), 0])
                proj_fill = None
                if p == NP_CORE - 1:
                    # out-proj tiles, gated on this pair's per-tqh epilogues
                    # (token quarter tt//4 is final once epilogue tqh=tt//4
                    # has been emitted at iter (tqh+1)*16; lag a few iters
                    # for the epilogue's DMA chain to land)
                    proj_fill = [gen_proj(), 32,
                                 lambda i: min((i // 16) * 8,
                                               max(0, i - 19)), 0]
                    fills.append(proj_fill)

                pending = None  # (tqh, kt, pt, psA, psB)
                psA = psB = None
                for i, (tqh, kt) in enumerate(ITERS):
                    st = stp.tile([128, 1024], F32, tag="st")
                    ktp = pair_kt[p]
                    qtp = pair_qt[p]
                    # same full-K stationary for both heads (zero-padded Q)
                    nc.tensor.matmul(
                        st[:, 0:512],
                        lhsT=ktp[:, kt * 128:(kt + 1) * 128],
                        rhs=qtp[:, 0, tqh * 512:(tqh + 1) * 512],
                        start=True, stop=True,
                    )
                    nc.tensor.matmul(
                        st[:, 512:1024],
                        lhsT=ktp[:, kt * 128:(kt + 1) * 128],
                        rhs=qtp[:, 1, tqh * 512:(tqh + 1) * 512],
                        start=True, stop=True,
                    )
                    pt = ptp.tile([128, 1024], BF16, tag="pt")
                    nc.scalar.activation(
                        out=pt, in_=st,
                        func=mybir.ActivationFunctionType.Exp, scale=SCALE,
                    )
                    if pending is not None:
                        ptqh, pkt, ppt, ppsA, ppsB = pending
                        emit_ot(p, pkt, ppt, ppsA, ppsB)
                        if pkt == KT_TILES - 1:
                            emit_epilogue(p, ptqh, ppsA, ppsB)
                    if kt == 0:
                        psA = otp.tile([128, 512], F32, tag="psA")
                        psB = otp.tile([128, 512], F32, tag="psB")
                    pending = (tqh, kt, pt, psA, psB)
                    # interleave fill ticks
                    for f in fills:
                        gen, total, due_fn, _t = f
                        due = min(total, due_fn(i))
                        while f[3] < due:
                            try:
                                next(gen)
                                f[3] += 1
                            except StopIteration:
                                f[3] = total
                                break
                ptqh, pkt, ppt, ppsA, ppsB = pending
                emit_ot(p, pkt, ppt, ppsA, ppsB)
                emit_epilogue(p, ptqh, ppsA, ppsB,
                              fast_tail=(p == NP_CORE - 1))

                # drain remaining fill work; the out-proj remainder is
                # emitted in split form so its pair-0..2 matmuls overlap
                # the final epilogue's chain
                for f in fills:
                    if f is proj_fill:
                        continue
                    for _ in f[0]:
                        pass
                if proj_fill is not None:
                    emit_tail_proj(proj_fill[3])

    _split_multi_waits(nc)
    return nc


_NC_CACHE = None


def _get_nc():
    global _NC_CACHE
    if _NC_CACHE is None:
        _NC_CACHE = _build_nc()
    return _NC_CACHE


# ---------------------------------------------------------------------------
# Host wrapper
# ---------------------------------------------------------------------------

def kernel(x, W_qkv, W_proj, cos, sin, mask):
    bf = ml_dtypes.bfloat16
    x = np.asarray(x, dtype=np.float32)
    W_qkv = np.asarray(W_qkv, dtype=np.float32)
    W_proj = np.asarray(W_proj, dtype=np.float32)
    cos = np.asarray(cos, dtype=np.float32)
    sin = np.asarray(sin, dtype=np.float32)

    # Permute q/k head dims: interleaved (x1,x2 pairs) -> halves [x1; x2].
    perm = np.concatenate([np.arange(0, HD, 2), np.arange(1, HD, 2)])
    Wq = W_qkv[0:C].reshape(H, HD, C)[:, perm, :].reshape(C, C)
    Wk = W_qkv[C:2 * C].reshape(H, HD, C)[:, perm, :].reshape(C, C)
    Wv = W_qkv[2 * C:3 * C]

    # per-pair tiled layouts: [NPAIR, 128 c-part, CC, 128 d]
    wqt = np.ascontiguousarray(
        Wq.T.astype(bf).reshape(CC, 128, NPAIR, 128).transpose(2, 1, 0, 3)
    )
    wkt = np.ascontiguousarray(
        Wk.T.astype(bf).reshape(CC, 128, NPAIR, 128).transpose(2, 1, 0, 3)
    )
    # V weights grouped by 8 heads (512 columns) for the N=512 streams
    wvt = np.ascontiguousarray(
        Wv.T.astype(bf).reshape(CC, 128, 2, 512).transpose(2, 1, 0, 3)
    )
    wpt = np.ascontiguousarray(W_proj.T.astype(bf))

    # RoPE tables in transposed/replicated layout:
    #   cosr[r, t] = cos[t, r % 32]
    #   sinB[r, t] = +sin[t, r%32] for (r%64)<32 else -sin[t, r%32]
    cosT = cos.T
    sinT = sin.T
    cosr = np.ascontiguousarray(np.tile(cosT, (4, 1)).astype(bf))
    sinB = np.ascontiguousarray(
        np.tile(np.concatenate([sinT, -sinT], axis=0), (2, 1)).astype(bf)
    )

    # Tensor-parallel head split: core c owns batch b = c//2 and head half
    # hf = c%2 (8 heads = 4 pairs), ALL 2048 q tokens.  K/V computed only
    # for the owned heads (no redundancy); the row-parallel out-proj yields
    # partial sums which the host adds while unsharding.
    in_maps = []
    xtb_cache = {}
    for c in range(NCORES):
        b, hf = divmod(c, 2)
        if b not in xtb_cache:
            xtb_cache[b] = np.ascontiguousarray(x[b].T.astype(bf))
        in_maps.append(
            {
                "xt": xtb_cache[b],
                "wqt": wqt[hf * NP_CORE:(hf + 1) * NP_CORE],
                "wkt": wkt[hf * NP_CORE:(hf + 1) * NP_CORE],
                "wvt": wvt[hf],
                "wpt": np.ascontiguousarray(
                    wpt[hf * (C // 2):(hf + 1) * (C // 2)]),
                "cosk": cosr,
                "sink": sinB,
            }
        )

    nc = _get_nc()
    trace = bool(int(os.environ.get("BASSK_TRACE", "0")))
    res = bass_utils.run_bass_kernel_spmd(
        nc, in_maps, core_ids=list(range(NCORES)), trace=trace
    )
    if trace:
        kernel.last_exec_time_ns = res.exec_time_ns
        kernel.last_profile = res

    # unshard: add the two head-halves' partial projections per batch
    out = np.empty((B, T, C), dtype=np.float32)
    for b in range(B):
        out[b] = res.results[2 * b]["out"]
        out[b] += res.results[2 * b + 1]["out"]
    return out
